# revision 17
# baseline (speedup 1.0000x reference)
"""Trainium2 Bass kernel for nn_PixelTransformer (v3).

Structure (see reference semantics; valid for any input values):
  * The transformer state is position-only (x enters only in the final
    flow); attention+residual folds to a per-layer 5x5 map; the state u
    carries the UNSCALED centered layer output with std folded forward
    (normalizers t1 = std(LN1 in), s = std(LN2 in) enter as rank-1 fp8
    DoubleRow bias pairs against a [t1|s] stack, as in v2).
  * Pixels split into two 64-wide streams A/B per core; each stream has
    its own psum banks / relu / variance chain so the two spines
    interleave on the engines (ACT: sqrts + one relu bank per stream;
    DVE: psum->bf16 copies, squares via bf16-SBUF 2x mode, the other
    relu bank; squares are NOT done on ACT to keep it under the relu
    budget).
  * Layer 0 is position-only with s0=1 and t1_0 host-precomputed, so
    its state is a 7-row constant [tokc; 1; t1_0] and wm0/R_aug0 absorb
    ALL layer-0 biases: no stack, no variance chain, no bias matmuls.
  * Head: biases via DoubleRow pairs against the [0|s8] stack; flow
    scan in closed form as in v2 (triangular matmul + full-sum row at
    partition 32).  NOTE: InstTensorTensorReduce and per-stream SLICED
    DoubleRow rhs pairs crash the exec unit on real TRN2 (CoreSim
    accepts both) -- stacks are stored as per-(layer,stream) contiguous
    [t1|s] 128-byte blocks and reductions use tensor_tensor +
    tensor_reduce instead.

Sharding: N=1024 pixels over 8 cores (128 each); weights replicated.
"""

import numpy as np

B, H, W = 32, 32, 32
N = H * W
L, D, FF = 8, 5, 2048
D0 = D + 2                # layer-0 state rows (u + s0 + t1_0)
NCORES = 8
NP = N // NCORES          # 128 pixels per core
NS = 2                    # streams per core
NPS = NP // NS            # 64 pixels per stream
NCHUNK = FF // 128        # 16
EPS = 1e-5

_PROG = None


def _build_program():
    import concourse.bacc as bacc
    import concourse.mybir as mybir
    import concourse.tile as tile

    f32 = mybir.dt.float32
    bf16 = mybir.dt.bfloat16
    fp8 = mybir.dt.float8e4
    AF = mybir.ActivationFunctionType
    ALU = mybir.AluOpType
    PM = mybir.MatmulPerfMode

    nc = bacc.Bacc(name="pixel_transformer3")

    # crit: [7, 416] bf16: tok0 0:128 (row5=1, row6=t1_0) | MT x8
    # 128:256 (rows 0:5) | R_augT x8 256:384 (l=0 rows 0:7) | H1T 384:400
    crit_d = nc.dram_tensor("crit", [D0, 416], bf16, kind="ExternalInput")
    wm_d = [nc.dram_tensor(f"wm{l}", [D0 if l == 0 else D, FF], bf16,
                           kind="ExternalInput") for l in range(L)]
    # w2p8: [128, 2048] fp8 mm2 DoubleRow weights (v2 layout)
    w2p_d = nc.dram_tensor("w2p8", [128, 2048], fp8, kind="ExternalInput")
    # per-layer fp8 pairs (l=1..7): FFN bias pairs [B1|W1u cc] 0:4096,
    # bCC pair 4096:4128, bR pair 4128:4160
    bff_d = [nc.dram_tensor(f"bff{l}", [1, 4160], fp8, kind="ExternalInput")
             for l in range(1, L)]
    headb_d = nc.dram_tensor("headb8", [1, 96], fp8, kind="ExternalInput")
    headw_d = nc.dram_tensor("headw", [16, 65], bf16, kind="ExternalInput")
    xsh_d = nc.dram_tensor("xsh", [B, NP], f32, kind="ExternalInput")
    out_d = nc.dram_tensor("out48", [48, 1], f32, kind="ExternalOutput")

    with tile.TileContext(nc) as tc:
        with (
            tc.tile_pool(name="consts", bufs=1) as cp,
            tc.tile_pool(name="work", bufs=3) as wp,
            tc.tile_pool(name="fsb", bufs=4) as fp,
            tc.tile_pool(name="ps", bufs=2, space="PSUM") as pp,
        ):
            # ---- input DMAs ----
            crit = cp.tile([D0, 416], bf16)
            wm = cp.tile([D0, L * FF], bf16)
            w2p = cp.tile([128, 2048], fp8)
            bff = cp.tile([1, 4160 * L], fp8)     # slot l at 4160*l, l>=1
            headb = cp.tile([1, 96], fp8)
            headw = cp.tile([16, 65], bf16)
            xsb = cp.tile([B, NP], f32)

            # Pool: wm0-bank1 (consumed first), bff1, wm1, bff3, wm3, ...
            # SP:   crit, wm0-bank0, w2p, bff2, wm2, bff4, wm4, ...
            nc.gpsimd.dma_start(out=wm[:, 1024:2048], in_=wm_d[0][:, 1024:2048])
            nc.sync.dma_start(out=crit, in_=crit_d[:, :])
            nc.gpsimd.dma_start(out=bff[0:1, 4160:8320], in_=bff_d[0][:, :])
            nc.sync.dma_start(out=wm[:, 0:1024], in_=wm_d[0][:, 0:1024])
            nc.sync.dma_start(out=w2p, in_=w2p_d[:, :])
            nc.gpsimd.dma_start(out=wm[0:D, FF:2 * FF], in_=wm_d[1][:, :])
            for l in range(2, L):
                q = nc.sync if l % 2 == 0 else nc.gpsimd
                q.dma_start(out=bff[0:1, 4160 * l:4160 * (l + 1)],
                            in_=bff_d[l - 1][:, :])
                q2 = nc.gpsimd if l % 2 == 0 else nc.sync
                q2.dma_start(out=wm[0:D, FF * l:FF * (l + 1)], in_=wm_d[l][:, :])
            nc.sync.dma_start(out=headb, in_=headb_d[:, :])
            nc.sync.dma_start(out=headw, in_=headw_d[:, :])
            nc.gpsimd.dma_start(out=xsb, in_=xsh_d[:, :])

            # ---- constants ----
            warmt = cp.tile([1, 1], f32)
            nc.vector.memset(warmt, 1.0)
            warmo = cp.tile([1, 1], f32)
            nc.scalar.activation(out=warmo, in_=warmt, func=AF.Sqrt)
            vconst = cp.tile([D, 1], bf16)
            nc.vector.memset(vconst, 1.0 / D)
            ones16c = cp.tile([1, 16], bf16)
            nc.vector.memset(ones16c, 1.0)
            ones16x32 = cp.tile([16, B], bf16)
            nc.vector.memset(ones16x32, 1.0)
            ones33 = cp.tile([33, B], bf16)
            nc.vector.memset(ones33, 1.0)
            # [t1|s] stacks l=1..7 + head [0|s8]; zeroed once (sqrt
            # writes fill the live slots; zero slots pair with zero
            # weights).
            stackbig = cp.tile([1, 256 * L], fp8)
            nc.vector.memset(stackbig, 0.0)

            tok0 = crit[:, 0:128]                  # [7, 128]
            MT = lambda l: crit[0:D, 128 + 16 * l:144 + 16 * l]
            RT = lambda l: crit[0:(D0 if l == 0 else D),
                                256 + 16 * l:272 + 16 * l]
            H1T = crit[0:D, 384:400]

            def stack_ap(l):
                # l in 1..7 -> [t1|s]; l == 8 -> head [0|s8]
                return stackbig[0:1, 256 * (l - 1):256 * l]

            def stkpair(l, s=None):
                pr = stack_ap(l).rearrange("p (two n) -> p two n", two=2)
                if s is None:
                    return pr
                return pr[:, :, NPS * s:NPS * (s + 1)]

            def bFF(l, c):
                a = bff[0:1, 4160 * l + 256 * c:4160 * l + 256 * (c + 1)]
                return a.rearrange("p (two m) -> p two m", two=2)

            def bCC(l):
                return bff[0:1, 4160 * l + 4096:4160 * l + 4128].rearrange(
                    "p (two m) -> p two m", two=2)

            def bR(l):
                return bff[0:1, 4160 * l + 4128:4160 * l + 4160].rearrange(
                    "p (two m) -> p two m", two=2)

            def bHead(k):
                return headb[0:1, 32 * k:32 * (k + 1)].rearrange(
                    "p (two m) -> p two m", two=2)

            def w2pair(l, p):
                return w2p[:, 256 * l + 32 * p:256 * l + 32 * (p + 1)].rearrange(
                    "p (two m) -> p two m", two=2)

            CHUNK_ORDER = list(range(8, 16)) + list(range(0, 8))

            u_cur = tok0              # layer-0 state [7, NP] bf16 const
            psy2_prev = [None, None]

            for l in range(L + 1):
                last = (l == L)
                u_next = None
                psyv = [None, None]
                psy2 = [None, None]
                if l > 0 and not last:
                    u_next = wp.tile([D, NP], bf16, tag="u", bufs=2,
                                     name=f"u{l}")
                elif last:
                    u_next = wp.tile([D, NP], bf16, tag="u", bufs=2,
                                     name="u8")
                for s in range(NS):
                    cols = slice(NPS * s, NPS * (s + 1))
                    # ---- boundary l-1 -> l, stream s ----
                    if l > 0:
                        psyv[s] = pp.tile([16, 512], f32, tag=f"psyv{s}",
                                          bufs=1, name=f"psyv{l}_{s}")
                        # DVE: u copy then square (bf16 SBUF, 2x mode)
                        nc.vector.tensor_copy(out=u_next[:, cols],
                                              in_=psy2_prev[s][0:D, 0:NPS])
                        sq2 = wp.tile([D, NPS], bf16, tag=f"sq2{s}",
                                      name=f"sq2_{l}_{s}")
                        nc.vector.tensor_tensor(out=sq2, in0=u_next[:, cols],
                                                in1=u_next[:, cols],
                                                op=ALU.mult)
                        psv2 = psyv[s][0:1, 128:192]
                        nc.tensor.matmul(psv2, vconst, sq2, start=True,
                                         stop=True, skip_group_check=True)
                        # ACT: sqrt2 -> stack(l) slot1 (fp8)
                        nc.scalar.activation(
                            out=stack_ap(l)[0:1, 128 + NPS * s:
                                            128 + NPS * (s + 1)],
                            in_=psv2, func=AF.Sqrt)
                    if last:
                        continue
                    ucs = u_cur if l == 0 else u_next
                    # ---- PE front: psy1, R-part, mains ----
                    psy2[s] = pp.tile([16, 512], f32, tag=f"psy2{s}",
                                      bufs=1, name=f"psy2_{l}_{s}")
                    if l > 0:
                        psy1 = psyv[s][0:16, 0:64]
                        nc.tensor.matmul(psy1, MT(l), ucs[:, cols],
                                         start=True, stop=False,
                                         skip_group_check=True)
                        nc.tensor.matmul(psy1, bCC(l), stkpair(l, s),
                                         start=False, stop=True,
                                         perf_mode=PM.DoubleRow,
                                         skip_group_check=True)
                    nc.tensor.matmul(psy2[s][0:16, 0:NPS], RT(l),
                                     ucs[:, cols], start=True, stop=False,
                                     skip_group_check=True)
                    psfs = [None, None]
                    for b in range(2):
                        psfs[b] = pp.tile([128, 512], f32, tag="pf",
                                          bufs=4, name=f"psf{l}_{s}{b}")
                    for c in CHUNK_ORDER:
                        b, c8 = c // 8, c % 8
                        nc.tensor.matmul(
                            psfs[b][:, 64 * c8:64 * (c8 + 1)],
                            wm[:, FF * l + 128 * c:FF * l + 128 * (c + 1)]
                            if l == 0 else
                            wm[0:D, FF * l + 128 * c:FF * l + 128 * (c + 1)],
                            ucs[:, cols],
                            start=(c8 == 0), stop=(l == 0 and c8 == 7),
                        )
                    # ---- variance chain for t1(l) (l=0: on host) ----
                    if l > 0:
                        y1b = wp.tile([D, NPS], bf16, tag=f"y1b{s}",
                                      name=f"y1b{l}_{s}")
                        nc.vector.tensor_copy(out=y1b,
                                              in_=psyv[s][0:D, 0:64])
                        sq1 = wp.tile([D, NPS], bf16, tag=f"sq1{s}",
                                      name=f"sq1_{l}_{s}")
                        nc.vector.tensor_tensor(out=sq1, in0=y1b, in1=y1b,
                                                op=ALU.mult)
                        psv1 = psyv[s][0:1, 64:128]
                        nc.tensor.matmul(psv1, vconst, sq1, start=True,
                                         stop=True, skip_group_check=True)
                        nc.scalar.activation(
                            out=stack_ap(l)[0:1, NPS * s:NPS * (s + 1)],
                            in_=psv1, func=AF.Sqrt)
                    # ---- FFN tail ----
                    if l > 0:
                        for c in CHUNK_ORDER:
                            b, c8 = c // 8, c % 8
                            nc.tensor.matmul(
                                psfs[b][:, 64 * c8:64 * (c8 + 1)],
                                bFF(l, c), stkpair(l, s),
                                start=False, stop=(c8 == 7),
                                perf_mode=PM.DoubleRow,
                            )
                        nc.tensor.matmul(psy2[s][0:16, 0:NPS],
                                         bR(l), stkpair(l, s), start=False,
                                         stop=False, perf_mode=PM.DoubleRow,
                                         skip_group_check=True)
                    # relu: bank1 on DVE, bank0 on ACT
                    f1 = fp.tile([128, 512], fp8, tag="f", name=f"f{l}_{s}1")
                    nc.vector.tensor_scalar(out=f1, in0=psfs[1],
                                            scalar1=0.0, scalar2=None,
                                            op0=ALU.max)
                    f0 = fp.tile([128, 512], fp8, tag="f", name=f"f{l}_{s}0")
                    nc.scalar.activation(out=f0, in_=psfs[0], func=AF.Relu)
                    for b, fq in ((1, f1), (0, f0)):
                        for j in range(4):
                            fpair = fq[:, 128 * j:128 * (j + 1)].rearrange(
                                "p (two n) -> p two n", two=2)
                            nc.tensor.matmul(
                                psy2[s][0:16, 0:NPS],
                                w2pair(l, 4 * b + j), fpair,
                                start=False,
                                stop=(b == 0 and j == 3),
                                perf_mode=PM.DoubleRow,
                                skip_group_check=True,
                            )
                if last:
                    break
                u_cur = u_next
                psy2_prev = psy2

            # ================= head =================
            u8 = u_next                      # [5, NP] bf16
            stk8 = stack_ap(L)               # [0|s8] fp8

            psh = pp.tile([16, NP], f32, tag="psy2", bufs=2, name="psh")
            nc.tensor.matmul(psh, H1T, u8[0:D, :], start=True, stop=False,
                             skip_group_check=True)
            nc.tensor.matmul(psh, bHead(0), stkpair(L), start=False,
                             stop=True, perf_mode=PM.DoubleRow,
                             skip_group_check=True)
            rec8 = wp.tile([1, NP], bf16, tag="rec8")
            with nc.allow_low_precision(reason="1/s8 feeds bf16 matmul"):
                nc.vector.reciprocal(out=rec8, in_=stk8[0:1, 128:256])
            hid = wp.tile([16, NP], bf16, tag="hid")
            nc.vector.tensor_scalar(out=hid, in0=psh, scalar1=0.0,
                                    scalar2=None, op0=ALU.max)
            r8bc = pp.tile([16, NP], f32, tag="psyv", bufs=2, name="r8bc")
            nc.tensor.matmul(r8bc, ones16c, rec8, start=True, stop=True,
                             skip_group_check=True)
            r8bcs = wp.tile([16, NP], bf16, tag="r8bcs")
            nc.vector.tensor_copy(out=r8bcs, in_=r8bc)

            psss = pp.tile([16, NP], f32, tag="psy2", bufs=2, name="psss")
            nc.tensor.matmul(psss, headw[:, 33:49], hid, start=True,
                             stop=False, skip_group_check=True)
            nc.tensor.matmul(psss, bHead(1), stkpair(L), start=False,
                             stop=True, perf_mode=PM.DoubleRow,
                             skip_group_check=True)
            psst = pp.tile([16, NP], f32, tag="psyv", bufs=2, name="psst")
            nc.tensor.matmul(psst, headw[:, 49:65], hid, start=True,
                             stop=False, skip_group_check=True)
            nc.tensor.matmul(psst, bHead(2), stkpair(L), start=False,
                             stop=True, perf_mode=PM.DoubleRow,
                             skip_group_check=True)

            # s_sb = psss * r8 with fused row-sum -> ssum (DMA'd early)
            s_sb = wp.tile([16, NP], f32, tag="s_sb")
            ssum = wp.tile([16, 1], f32, tag="ssum")
            nc.vector.tensor_tensor_reduce(out=s_sb, in0=psss, in1=r8bcs,
                                           scale=1.0, scalar=0.0,
                                           op0=ALU.mult, op1=ALU.add,
                                           accum_out=ssum)
            nc.sync.dma_start(out=out_d[0:16, :], in_=ssum)
            t_sb = wp.tile([16, NP], f32, tag="t_sb")
            nc.vector.tensor_tensor(out=t_sb, in0=psst, in1=r8bcs,
                                    op=ALU.mult)

            th = wp.tile([16, NP], bf16, tag="th")
            nc.scalar.activation(out=th, in_=s_sb, func=AF.Tanh)
            psD = pp.tile([33, NP], f32, tag="pf", bufs=4, name="psD")
            nc.tensor.matmul(psD, headw[:, 0:33], th, start=True, stop=True)
            wexp = wp.tile([33, NP], bf16, tag="wexp")
            nc.scalar.activation(out=wexp, in_=psD, func=AF.Exp)
            wt = wp.tile([16, NP], bf16, tag="wt")
            nc.gpsimd.tensor_tensor(out=wt, in0=wexp[0:16, :], in1=t_sb,
                                    op=ALU.mult)
            pscb = pp.tile([B, NP], f32, tag="pf", bufs=4, name="pscb")
            nc.tensor.matmul(pscb, ones16x32, wt, start=True, stop=True)
            pseb = pp.tile([B, NP], f32, tag="pf", bufs=4, name="pseb")
            nc.tensor.matmul(pseb, ones33[32:33, :], wexp[32:33, :],
                             start=True, stop=True)
            zt = wp.tile([B, NP], f32, tag="zt")
            nc.vector.tensor_tensor(out=zt, in0=xsb, in1=pseb, op=ALU.mult)
            z = wp.tile([B, NP], f32, tag="z")
            nc.vector.tensor_tensor(out=z, in0=zt, in1=pscb, op=ALU.add)
            zscr = wp.tile([B, NP], f32, tag="zscr")
            zr = wp.tile([B, 1], f32, tag="zr")
            nc.vector.tensor_tensor_reduce(out=zscr, in0=z, in1=z,
                                           scale=1.0, scalar=0.0,
                                           op0=ALU.mult, op1=ALU.add,
                                           accum_out=zr)
            nc.sync.dma_start(out=out_d[16:48, :], in_=zr)

    nc.finalize()
    return nc


def _fold_inputs(inp):
    """Host-side weight folding (float64, cast at the end)."""
    import ml_dtypes

    f8 = ml_dtypes.float8_e4m3fn
    C = np.eye(D) - np.ones((D, D)) / D
    g = lambda k: np.asarray(inp[k], dtype=np.float64)
    wqkv, bqkv, wo, bo = g("wqkv"), g("bqkv"), g("wo"), g("bo")
    w1, b1, w2, b2 = g("w1"), g("b1"), g("w2"), g("b2")
    ln1w, ln1b, ln2w, ln2b = g("ln1w"), g("ln1b"), g("ln2w"), g("ln2b")

    t0 = np.array([-1.0, 0.0, 1.0, 0.0, 1.0])

    crit = np.zeros((D0, 416), np.float64)
    wm = np.zeros((D0, L * FF), np.float64)   # rows 5:7 used only for l=0
    w2p = np.zeros((128, 2048), np.float64)
    bffs = np.zeros((L, 4160), np.float64)    # l=0 slot unused
    Ms, ccs = [], []

    for l in range(L):
        Dl = np.diag(ln2w[l - 1]) if l > 0 else np.eye(D)
        el = ln2b[l - 1] if l > 0 else np.zeros(D)
        wv = wqkv[l][2 * D:3 * D, :]
        bv = bqkv[l][2 * D:3 * D]
        A0 = np.eye(D) + wo[l] @ wv
        ca = wo[l] @ bv + bo[l]
        M = C @ A0 @ Dl
        cc = C @ (A0 @ el + ca)
        if l == 0:
            cc = cc + C @ (A0 @ t0)
        Ms.append(M)
        ccs.append(cc)
        W1u = w1[l] * ln1w[l][None, :]          # [FF, 5]
        B1 = b1[l] + w1[l] @ ln1b[l]            # [FF]
        G1 = W1u @ M                            # [FF, 5]
        Gs = W1u @ cc                           # [FF]  (x s)
        R = C @ np.diag(ln1w[l]) @ M
        Rs = C @ np.diag(ln1w[l]) @ cc          # (x s)
        bRt = C @ (ln1b[l] + b2[l])             # (x t1)

        crit[0:D, 256 + 16 * l:261 + 16 * l] = R.T
        wm[0:D, FF * l:FF * (l + 1)] = G1.T
        if l == 0:
            # layer-0 state rows: [u; s0=1; t1_0] -> fold s- and
            # t1-biases directly into the matmul weights
            wm[D, 0:FF] = Gs
            wm[D + 1, 0:FF] = B1
            crit[D, 256:261] = Rs
            crit[D + 1, 256:261] = bRt
        else:
            crit[0:D, 128 + 16 * l:133 + 16 * l] = M.T
            for c in range(NCHUNK):
                bffs[l, 256 * c:256 * c + 128] = B1[128 * c:128 * (c + 1)]
                bffs[l, 256 * c + 128:256 * (c + 1)] = Gs[128 * c:
                                                          128 * (c + 1)]
            bffs[l, 4112:4117] = cc             # bCC pair slot1
            bffs[l, 4128:4133] = bRt            # bR pair slot0
            bffs[l, 4144:4149] = Rs             # bR pair slot1

        W2c = C @ w2[l]                         # [5, FF]
        for p in range(8):
            for i in range(2):
                c = 2 * p + i
                base = 256 * l + 32 * p + 16 * i
                w2p[:, base:base + 5] = W2c[:, 128 * c:128 * (c + 1)].T

    # head folds
    f0w1, f0b1 = g("f0w1"), g("f0b1")
    f0w2, f0b2 = g("f0w2"), g("f0b2")
    D8 = np.diag(ln2w[L - 1])
    e8 = ln2b[L - 1]
    sf = float(np.exp(np.asarray(inp["sfac"], dtype=np.float64)[0]))
    H1 = f0w1 @ D8                              # [16, 5]
    hb1 = f0b1 + f0w1 @ e8
    crit[0:D, 384:400] = H1.T
    headb = np.zeros((1, 96), np.float64)
    headb[0, 16:32] = hb1                       # psh pair slot1
    headb[0, 48:64] = f0b2[0:16] / sf           # psss pair slot1
    headb[0, 80:96] = f0b2[16:32]               # psst pair slot1

    headw = np.zeros((16, 65), np.float64)
    for j in range(16):
        headw[j + 1:16, j] = sf                 # TRIext cols 0..15
    headw[:, 32] = sf                           # full-sum col (partition 32)
    headw[:, 33:49] = (f0w2[0:16, :] / sf).T
    headw[:, 49:65] = f0w2[16:32, :].T

    # positional tokens (fp32 ops to match reference), centered by t0
    xs = (np.arange(W, dtype=np.float32) / np.float32(1e4)).astype(np.float32)
    ys = (np.arange(H, dtype=np.float32) / np.float32(1e4)).astype(np.float32)
    sinx = np.broadcast_to(np.sin(xs)[None, :], (H, W)).reshape(N)
    cosx = np.broadcast_to(np.cos(xs)[None, :], (H, W)).reshape(N)
    siny = np.broadcast_to(np.sin(ys)[:, None], (H, W)).reshape(N)
    cosy = np.broadcast_to(np.cos(ys)[:, None], (H, W)).reshape(N)
    tok = np.stack(
        [-np.ones(N, np.float32), sinx, cosx, siny, cosy], axis=0
    ).astype(np.float64)
    tokc = tok - t0[:, None]                    # [5, N] tiny values
    xflat = np.asarray(inp["x"], dtype=np.float32)[:, 0].reshape(B, N)

    # layer-0 std (position-only): y1_0 = M0 tokc + cc0, t1_0 = rms
    y1_0 = Ms[0] @ tokc + ccs[0][:, None]       # [5, N]
    t1_0 = np.sqrt((y1_0 ** 2).mean(axis=0) + EPS)  # [N]

    out = {
        "crit": crit.astype(ml_dtypes.bfloat16),
        "w2p8": w2p.astype(f8),
        "headw": headw.astype(ml_dtypes.bfloat16),
        "headb8": headb.astype(f8),
        "tokc": tokc.astype(ml_dtypes.bfloat16),
        "t1_0": t1_0,
        "xsh": xflat,
        "sf": sf,
    }
    for l in range(L):
        rows = D0 if l == 0 else D
        out[f"wm{l}"] = np.ascontiguousarray(
            wm[0:rows, FF * l:FF * (l + 1)]).astype(ml_dtypes.bfloat16)
    for l in range(1, L):
        out[f"bff{l}"] = np.ascontiguousarray(
            bffs[l:l + 1, :]).astype(f8)
    return out


def get_program():
    global _PROG
    if _PROG is None:
        _PROG = _build_program()
    return _PROG


def make_in_maps(inputs):
    import ml_dtypes

    arrs = _fold_inputs(inputs)
    shared_keys = (["w2p8", "headw", "headb8"]
                   + [f"wm{l}" for l in range(L)]
                   + [f"bff{l}" for l in range(1, L)])
    shared = {k: arrs[k] for k in shared_keys}
    in_maps = []
    for core in range(NCORES):
        sl = slice(core * NP, (core + 1) * NP)
        m = dict(shared)
        crit = arrs["crit"].copy()
        crit[0:D, 0:128] = arrs["tokc"][:, sl]
        crit[D, 0:128] = 1.0
        crit[D + 1, 0:128] = arrs["t1_0"][sl].astype(ml_dtypes.bfloat16)
        m["crit"] = np.ascontiguousarray(crit)
        m["xsh"] = np.ascontiguousarray(arrs["xsh"][:, sl])
        in_maps.append(m)
    return in_maps, arrs["sf"]


def combine_outputs(outs, sf):
    """per-core [48,1]: rows 0:16 = ssum (s_/sf), rows 16:48 = z^2 sums."""
    s_tot = 0.0
    q_tot = 0.0
    for o in outs:
        o = np.asarray(o, dtype=np.float64).reshape(48)
        s_tot += o[0:16].sum() * sf
        q_tot += o[16:48].sum()
    sldj = B * s_tot - 0.5 * q_tot - B * N * 0.5 * np.log(2.0 * np.pi)
    return np.array(-sldj, dtype=np.float32)


def kernel(**inputs):
    from concourse.bass_utils import run_bass_kernel_spmd

    nc = get_program()
    in_maps, sf = make_in_maps(inputs)
    res = run_bass_kernel_spmd(nc, in_maps, core_ids=list(range(NCORES)))
    return combine_outputs([r["out48"] for r in res.results], sf)
